# revision 9
# baseline (speedup 1.0000x reference)
"""Trainium2 Bass kernel: ActiveBlockPromptBasis (moe_routing), v2.

Math (per batch image b):
  pg  = wa.T @ xf + wb.T @ fb                      # [14, pix]
  g   = gelu(pg + gbias)                           # [14, pix]  (ACT bias)
  z   = mz.T @ g                                   # [48, pix]
  wun = exp(z + bz)                                # [48, pix]
  S48 = ones[48,48].T @ wun                        # column sum BROADCAST to
                                                   # all 48 partitions by the
                                                   # all-ones stationary
  wgt = wun * recip(S48)                           # softmax weights
  out = conv3x3(W'', wgt)                          # prompt matrix folded into
                                                   # the conv weights on host:
                                                   # W''[c,o] = sum_e W[o,e]*pt[c,e]

Conv: the wgt strip is stored twice in one [96, h, 257] SBUF tile:
  A (partitions 0-47):  A[c] = wgt[c-1]  (zero col at c=0)
  B (partitions 48-95): B[c] = wgt[c+1]  (zero col at c=255)
so taps (ky,0)+(ky,2) run as ONE 96-contraction matmul over cols 0..255
and tap (ky,1) reads A cols 1..256 with 48-contraction: 6 matmuls per
2-row output pair instead of 9, all f32-accumulating into one PSUM bank.

All activations/inputs in bf16 (1 cyc/col PE rate, half DMA traffic);
output drained f32->bf16 and converted to f32 on host. End-to-end rel
err vs the fp32 reference ~3e-3 (tolerance 2e-2).

Sharding: data-parallel over batch, one image per NeuronCore (8 cores).
"""

import numpy as np
from collections import deque
from contextlib import ExitStack

import concourse.bass as bass
import concourse.tile as tile
from concourse import bacc, mybir
from concourse.bass_utils import run_bass_kernel_spmd

F32 = mybir.dt.float32
F32R = mybir.dt.float32r
BF16 = mybir.dt.bfloat16
AFT = mybir.ActivationFunctionType

B, DIM, E = 8, 64, 128
NT, NB = 6, 8
NTK = NT * NB  # 48
NCORES = 8
W_ = 256
SCROWS = 4          # rows per superchunk
SCPIX = SCROWS * W_  # 1024
PHASE = 8           # superchunks per ACT-table phase


def build_program(h=256, w=256):
    assert w == 256 and h % SCROWS == 0
    PIX = h * w
    NSC = h // SCROWS
    PITCH = w + 1          # A uses cols 0..256, B uses cols 0..255
    mul = mybir.AluOpType.mult

    nc = bacc.Bacc("TRN2", target_bir_lowering=False, debug=False,
                   enable_asserts=False)

    xf_d = nc.dram_tensor("xf", [128, PIX], BF16, kind="ExternalInput")
    fb_d = nc.dram_tensor("fb", [64, PIX], BF16, kind="ExternalInput")
    wa_d = nc.dram_tensor("wa", [128, 14], BF16, kind="ExternalInput")
    wb_d = nc.dram_tensor("wb", [64, 14], BF16, kind="ExternalInput")
    gb_d = nc.dram_tensor("gb", [14, 1], F32, kind="ExternalInput")
    mz_d = nc.dram_tensor("mz", [14, NTK], BF16, kind="ExternalInput")
    bz_d = nc.dram_tensor("bz", [NTK, 1], F32, kind="ExternalInput")
    on48_d = nc.dram_tensor("on48", [NTK, NTK], F32R,
                            kind="ExternalInput")
    wpair_d = nc.dram_tensor("wpair", [112, 3 * E], BF16,
                             kind="ExternalInput")
    wcent_d = nc.dram_tensor("wcent", [NTK, 3 * E], BF16, kind="ExternalInput")
    out_d = nc.dram_tensor("out", [E, PIX], BF16, kind="ExternalOutput")

    with tile.TileContext(nc) as tc, ExitStack() as ctx:
        consts = ctx.enter_context(tc.tile_pool(name="consts", bufs=1))
        pxf = ctx.enter_context(tc.tile_pool(name="pxf", bufs=3))
        pfb = ctx.enter_context(tc.tile_pool(name="pfb", bufs=3))
        pg_sb = ctx.enter_context(tc.tile_pool(name="pgsb", bufs=10))
        pwun = ctx.enter_context(tc.tile_pool(name="pwun", bufs=3))
        prb = ctx.enter_context(tc.tile_pool(name="prb", bufs=2))
        pout = ctx.enter_context(tc.tile_pool(name="pout", bufs=2))
        pp_pg = ctx.enter_context(
            tc.tile_pool(name="pp_pg", bufs=1, space="PSUM"))
        pp_z = ctx.enter_context(
            tc.tile_pool(name="pp_z", bufs=1, space="PSUM"))
        pp_s = ctx.enter_context(
            tc.tile_pool(name="pp_s", bufs=1, space="PSUM"))
        pp_cv = ctx.enter_context(
            tc.tile_pool(name="pp_cv", bufs=2, space="PSUM"))

        # --- constants ---
        wa_sb = consts.tile([128, 14], BF16)
        nc.sync.dma_start(out=wa_sb[:], in_=wa_d[:])
        wb_sb = consts.tile([64, 14], BF16)
        nc.sync.dma_start(out=wb_sb[:], in_=wb_d[:])
        gb_sb = consts.tile([14, 1], F32)
        nc.sync.dma_start(out=gb_sb[:], in_=gb_d[:])
        mz_sb = consts.tile([14, NTK], BF16)
        nc.sync.dma_start(out=mz_sb[:], in_=mz_d[:])
        bz_sb = consts.tile([NTK, 1], F32)
        nc.sync.dma_start(out=bz_sb[:], in_=bz_d[:])
        on48_sb = consts.tile([NTK, NTK], F32R)
        nc.sync.dma_start(out=on48_sb[:], in_=on48_d[:])
        wpair_sb = consts.tile([112, 3 * E], BF16)
        nc.sync.dma_start(out=wpair_sb[:], in_=wpair_d[:])
        wcent_sb = consts.tile([NTK, 3 * E], BF16)
        nc.sync.dma_start(out=wcent_sb[:], in_=wcent_d[:])

        # whole-image wgt strip: A (parts 0-47) | dead zeros (48-63,
        # engine APs must start at partition 0/32/64/96) | B (parts 64-111)
        strip = consts.tile([112, h * PITCH], BF16)
        S3 = strip[:].rearrange("p (r c) -> p r c", c=PITCH)
        nc.vector.memset(S3[0:48, :, 0:1], 0.0)        # A zero col (kx=0 pad)
        for r0 in range(0, h, 128):                    # dead band (48-63);
            nc.gpsimd.memset(S3[32:64, r0:r0 + 128, :], 0.0)  # split: 16-bit
            # num_elem ISA field caps a single memset at 65535 elements
        nc.vector.memset(S3[64:112, :, w - 1:w], 0.0)  # B zero col (kx=2 pad)

        convq = deque()
        state = {"xf": {}, "fb": {}, "g": {}, "wun": {},
                 "stg": None, "last_exp": None, "first_gelu": None}

        def dma_in(i):
            if i >= NSC:
                return
            off = i * SCPIX
            xt = pxf.tile([128, SCPIX], BF16, tag="xf")
            nc.sync.dma_start(out=xt[:], in_=xf_d[:, off:off + SCPIX])
            ft = pfb.tile([64, SCPIX], BF16, tag="fb")
            nc.sync.dma_start(out=ft[:], in_=fb_d[:, off:off + SCPIX])
            state["xf"][i] = xt
            state["fb"][i] = ft

        def emit_pg(i):
            pg = pp_pg.tile([14, SCPIX], F32, tag="pg")
            xt, ft = state["xf"].pop(i), state["fb"].pop(i)
            for c in (0, 1):
                nc.tensor.matmul(pg[:, c * 512:(c + 1) * 512], wa_sb[:],
                                 xt[:, c * 512:(c + 1) * 512],
                                 start=True, stop=False)
            for c in (0, 1):
                nc.tensor.matmul(pg[:, c * 512:(c + 1) * 512], wb_sb[:],
                                 ft[:, c * 512:(c + 1) * 512],
                                 start=False, stop=True)
            return pg

        def emit_gelu(i, pg):
            g = pg_sb.tile([14, SCPIX], BF16, tag="g")
            for c in (0, 1):
                inst = nc.scalar.activation(
                    g[:, c * 512:(c + 1) * 512], pg[:, c * 512:(c + 1) * 512],
                    AFT.Gelu, bias=gb_sb[:])
                if state["first_gelu"] is None:
                    state["first_gelu"] = inst
            state["g"][i] = g

        def emit_z(i):
            z = pp_z.tile([NTK, SCPIX], F32, tag="z")
            g = state["g"].pop(i)
            for c in (0, 1):
                nc.tensor.matmul(z[:, c * 512:(c + 1) * 512], mz_sb[:],
                                 g[:, c * 512:(c + 1) * 512])
            return z

        def emit_exp(i, z):
            wun = pwun.tile([NTK, SCPIX], F32R, tag="wun")
            for c in (0, 1):
                state["last_exp"] = nc.scalar.activation(
                    wun[:, c * 512:(c + 1) * 512],
                    z[:, c * 512:(c + 1) * 512], AFT.Exp, bias=bz_sb[:])
            state["wun"][i] = wun

        def emit_colsum(i):
            s48 = pp_s.tile([NTK, SCPIX], F32, tag="s48")
            wun = state["wun"][i]
            for c in (0, 1):
                nc.tensor.matmul(s48[:, c * 512:(c + 1) * 512], on48_sb[:],
                                 wun[:, c * 512:(c + 1) * 512])
            return s48

        def emit_norm(i, s48):
            rb = prb.tile([NTK, SCPIX], F32, tag="rb")
            for c in (0, 1):
                nc.vector.reciprocal_approx_fast(
                    rb[:, c * 512:(c + 1) * 512],
                    s48[:, c * 512:(c + 1) * 512])
            wun = state["wun"].pop(i)
            y0 = i * SCROWS
            wun3 = wun[:].bitcast(F32).rearrange("p (r c) -> p r c", c=w)
            rb3 = rb[:].rearrange("p (r c) -> p r c", c=w)
            nc.vector.tensor_mul(
                S3[0:48, y0:y0 + SCROWS, 1:1 + w], wun3, rb3)
            nc.gpsimd.tensor_mul(
                S3[64:112, y0:y0 + SCROWS, 0:w - 1],
                wun3[:, :, 1:w], rb3[:, :, 1:w])
            # pairs that became fully normalized
            for yA in (4 * i - 2, 4 * i):
                if 0 <= yA <= h - 2:
                    convq.append(yA)
            if i == NSC - 1:
                convq.append(4 * i + 2)

        def emit_pair(yA):
            pcv = pp_cv.tile([128, 512], F32, tag="cv")
            mms = []
            for ky in (1, 0, 2):
                rlo = yA if yA + ky - 1 >= 0 else yA + 1
                rhi = yA + 1 if yA + ky <= h - 1 else yA
                mms.append((ky, rlo, rhi, "P"))
                mms.append((ky, rlo, rhi, "C"))
            for ti, (ky, rlo, rhi, mode) in enumerate(mms):
                nr = rhi - rlo + 1
                r0 = rlo + ky - 1
                out_ap = pcv[:, (rlo - yA) * w:(rhi - yA + 1) * w]
                if mode == "P":
                    st = wpair_sb[:, ky * E:(ky + 1) * E]
                    mv = S3[0:112, r0:r0 + nr, 0:w]
                else:
                    st = wcent_sb[:, ky * E:(ky + 1) * E]
                    mv = S3[0:48, r0:r0 + nr, 1:1 + w]
                nc.tensor.matmul(out_ap, st, mv,
                                 start=(ti == 0), stop=(ti == len(mms) - 1))
            # drain into 8-row staging; DMA out when full
            q = (yA // 2) % 4
            if q == 0:
                state["stg"] = pout.tile([128, 4 * 512], BF16, tag="stg",
                                         name="stg")
            dst = state["stg"][:, q * 512:(q + 1) * 512]
            nc.vector.tensor_copy(dst, pcv[:])
            if q == 3:
                g0 = yA - 6
                nc.gpsimd.dma_start(out=out_d[:, g0 * w:(g0 + 8) * w],
                                    in_=state["stg"][:])

        def conv_fill(nmax, reserve):
            n = 0
            while n < nmax and len(convq) > reserve:
                emit_pair(convq.popleft())
                n += 1

        # prologue
        dma_in(0)
        dma_in(1)
        phases = [list(range(p0, min(p0 + PHASE, NSC)))
                  for p0 in range(0, NSC, PHASE)]
        for scs in phases:
            # gelu phase
            state["first_gelu"] = None
            for i in scs:
                dma_in(i + 2)
                pg = emit_pg(i)
                emit_gelu(i, pg)
                if (state["last_exp"] is not None
                        and state["first_gelu"] is not None):
                    bass._add_dep_helper(
                        state["first_gelu"].ins, state["last_exp"].ins,
                        sync=True, reason="act-table-phase-order")
                    state["last_exp"] = None
                conv_fill(1, 2)
            # exp/norm phase
            for i in scs:
                z = emit_z(i)
                emit_exp(i, z)
                conv_fill(1, 0)
                s48 = emit_colsum(i)
                emit_norm(i, s48)
                conv_fill(1, 6)
        while convq:
            emit_pair(convq.popleft())

    nc.compile()
    return nc


_cache = {}


def get_program(h=256, w=256):
    key = (h, w)
    if key not in _cache:
        _cache[key] = build_program(h, w)
    return _cache[key]


def make_weight_inputs(prompt, conv_w, b_fc1_w, b_fc1_b, b_fc2_w, b_fc2_b,
                       t_fc1_w, t_fc1_b, t_fc2_w, t_fc2_b):
    import ml_dtypes
    f = np.float32
    bf = ml_dtypes.bfloat16
    wa = np.zeros((128, 14), f)
    wa[:64, :8] = b_fc1_w.T
    wa[64:128, 8:14] = t_fc1_w[:, :64].T
    wb = np.zeros((64, 14), f)
    wb[:, 8:14] = t_fc1_w[:, 64:].T
    gb = np.concatenate([b_fc1_b, t_fc1_b]).astype(f).reshape(14, 1)
    mz = np.zeros((14, NTK), f)
    bz = np.zeros((NTK, 1), f)
    for t in range(NT):
        for k in range(NB):
            c = t * NB + k
            mz[:8, c] = b_fc2_w[k, :]
            mz[8:, c] = t_fc2_w[t, :]
            bz[c, 0] = b_fc2_b[k] + t_fc2_b[t]
    pt = prompt.reshape(NTK, E).astype(f)
    # folded conv weights: w2[ky,kx][c,o] = sum_e conv_w[o,e,ky,kx]*pt[c,e]
    # wpair rows: 0-47 tap kx=0, 48-63 zero (dead band), 64-111 tap kx=2
    wpair = np.zeros((112, 3 * E), f)
    wcent = np.zeros((NTK, 3 * E), f)
    for ky in range(3):
        wpair[:NTK, ky * E:(ky + 1) * E] = pt @ conv_w[:, :, ky, 0].T
        wpair[64:, ky * E:(ky + 1) * E] = pt @ conv_w[:, :, ky, 2].T
        wcent[:, ky * E:(ky + 1) * E] = pt @ conv_w[:, :, ky, 1].T
    return {
        "wa": wa.astype(bf), "wb": wb.astype(bf), "gb": gb,
        "mz": mz.astype(bf), "bz": bz,
        "on48": np.ones((NTK, NTK), f),
        "wpair": wpair.astype(bf), "wcent": wcent.astype(bf),
    }


def make_core_inputs(x_b, flux_b, weights, h, w):
    import ml_dtypes
    bf = ml_dtypes.bfloat16
    PIX = h * w
    xf = np.concatenate(
        [x_b.reshape(DIM, PIX), flux_b[:64].reshape(64, PIX)],
        axis=0).astype(bf)
    fb = flux_b[64:].reshape(64, PIX).astype(bf)
    m = {"xf": np.ascontiguousarray(xf), "fb": np.ascontiguousarray(fb)}
    m.update(weights)
    return m


def kernel(x, flux, prompt, conv_w, b_fc1_w, b_fc1_b, b_fc2_w, b_fc2_b,
           t_fc1_w, t_fc1_b, t_fc2_w, t_fc2_b):
    x = np.asarray(x, np.float32)
    flux = np.asarray(flux, np.float32)
    flux = np.where(np.isnan(flux), np.float32(0), flux)
    h, w = x.shape[2], x.shape[3]

    nc = get_program(h=h, w=w)
    weights = make_weight_inputs(
        np.asarray(prompt, np.float32), np.asarray(conv_w, np.float32),
        np.asarray(b_fc1_w, np.float32), np.asarray(b_fc1_b, np.float32),
        np.asarray(b_fc2_w, np.float32), np.asarray(b_fc2_b, np.float32),
        np.asarray(t_fc1_w, np.float32), np.asarray(t_fc1_b, np.float32),
        np.asarray(t_fc2_w, np.float32), np.asarray(t_fc2_b, np.float32))
    in_maps = [make_core_inputs(x[i], flux[i], weights, h, w)
               for i in range(NCORES)]
    res = run_bass_kernel_spmd(nc, in_maps, list(range(NCORES)))
    out = np.stack(
        [np.asarray(res.results[i]["out"]).astype(np.float32).reshape(E, h, w)
         for i in range(NCORES)], axis=0)
    return out
